# revision 17
# baseline (speedup 1.0000x reference)
# GQA attention block on 8 Trainium2 NeuronCores.
# Sharding: core = (batch b in {0,1}) x (tensor-parallel t in {0..3}).
# Each core: batch row b, 4 query heads {4t..4t+3}, 2 kv heads {2t, 2t+1}.
# W_Q/W_K/W_V split column-wise (per-head), W_O row-wise; the 4 TP partial
# outputs per batch are summed on the host (the "all-reduce").
#
# v2: fp8 DoubleRow Q/K projections (2x MACs/cycle), paired-head attention
# (heads sharing a kv head share the K/V ldweights and a single 1024-wide
# exp activation), DMA issue order front-loads the Q/K operands, out-proj
# writes 4-bank PSUM tiles drained by single wide copies.
import math
import sys

sys.path.insert(0, "/opt/trn_rl_repo")

import ml_dtypes
import numpy as np

import concourse.bacc as bacc
import concourse.bass as bass
import concourse.mybir as mybir
import concourse.tile as tile
from contextlib import ExitStack

BF = mybir.dt.bfloat16
F32 = mybir.dt.float32
FP8 = mybir.dt.float8e4
bfnp = ml_dtypes.bfloat16
f8np = mybir.dt.np(FP8)

EMB = 2048
HEADS = 16
G = 2
HD = 128          # head dim
KV = HEADS // G   # 8 kv heads
B = 2
S = 2048
NCORES = 8
TP = 4
HQ = HEADS // TP       # 4 q heads per core
HKV = KV // TP         # 2 kv heads per core
NE = EMB // 128        # 16 contraction chunks
NE2 = EMB // 256       # 8 fp8 DoubleRow chunks
SC4 = S // 512         # 4 s-chunks of 512
SC16 = S // 128        # 16 s-chunks of 128
QKW = (HQ + HKV) * HD  # 768 combined q+k output cols
SCALE = 1.0 / math.sqrt(float(EMB))

USE_FP8_QK = False  # measured: fp8 Q/K alone costs 2.1% rel err (budget 2e-2)

_NC = None


def _build_program(loop_n=None, use_fp8=USE_FP8_QK):
    nc = bacc.Bacc("TRN2", target_bir_lowering=False, debug=False)

    xT = nc.dram_tensor("xT", (EMB, S), BF, kind="ExternalInput")
    if use_fp8:
        x8T = nc.dram_tensor("x8T", (EMB, S), FP8, kind="ExternalInput")
        wqk = nc.dram_tensor("wqk", (EMB, QKW), FP8, kind="ExternalInput")
    else:
        wqk = nc.dram_tensor("wqk", (EMB, QKW), BF, kind="ExternalInput")
    wv = nc.dram_tensor("wv", (EMB, HKV * HD), BF, kind="ExternalInput")
    wo = nc.dram_tensor("wo", (HQ * HD, EMB), BF, kind="ExternalInput")
    cosT = nc.dram_tensor("cosT", (HD, S), BF, kind="ExternalInput")
    sinT = nc.dram_tensor("sinT", (HD, S), BF, kind="ExternalInput")
    out = nc.dram_tensor("out", (S, EMB), BF, kind="ExternalOutput")

    DR = mybir.MatmulPerfMode.DoubleRow

    with tile.TileContext(nc) as tc, ExitStack() as ctx:
        persist = ctx.enter_context(tc.tile_pool(name="persist", bufs=1))
        # qk_sb j-blocks: 0..3 = roped Q heads, 4..5 = roped K kv-heads; [d, s]
        qk_sb = persist.tile([128, HQ + HKV, S], BF)
        # V in [t, d] layout: [t_part, t_chunk, kvl*128+d]
        v_sb = persist.tile([128, SC16, HKV * HD], BF)
        ctx_sb = persist.tile([128, HQ, S], BF)      # [d, head, s]
        wo_sb = persist.tile([128, HQ, EMB], BF)     # [d, head, e_out]
        cos_sb = persist.tile([128, S], BF)
        sin_sb = persist.tile([128, S], BF)
        ones_sb = persist.tile([128, 1], BF)
        warm_a = persist.tile([1, 8], F32)
        warm_b = persist.tile([1, 8], F32)
        nc.vector.memset(ones_sb, 1.0)
        nc.vector.memset(warm_a, 0.0)

        def _phases():
            # preload the exp table set while DMAs stream
            nc.scalar.activation(warm_b, warm_a, mybir.ActivationFunctionType.Exp)

            # ---------------- Phase 1: projections + RoPE ----------------
            with tc.tile_pool(name="xt", bufs=1) as xt_pool, \
                 tc.tile_pool(name="wts", bufs=1) as w_pool, \
                 tc.tile_pool(name="ropet", bufs=3) as rope_t, \
                 tc.tile_pool(name="pproj", bufs=8, space=bass.MemorySpace.PSUM) as pp:
                xt_sb = xt_pool.tile([128, NE, S], BF)
                if use_fp8:
                    x8_sb = xt_pool.tile([128, NE2, 2, S], FP8)
                    wqk_sb = w_pool.tile([128, NE2, 2, QKW], FP8)
                else:
                    wqk_sb = w_pool.tile([128, NE, QKW], BF)
                wv_sb = w_pool.tile([128, NE, HKV * HD], BF)

                # DMA order (the queue is FIFO, so order = priority):
                # chunk-0 q/k operands, cos/sin, remaining q/k chunks.
                # wv is emitted after the K pair, wo after all projections,
                # and each rope's swap DMAs interleave naturally.
                def dma_qk_chunk(c):
                    if use_fp8:
                        c2, ko = divmod(c, 2)
                        nc.sync.dma_start(out=wqk_sb[:, c2, ko, :],
                                          in_=wqk[c * 128:(c + 1) * 128, :])
                        nc.sync.dma_start(out=x8_sb[:, c2, ko, :],
                                          in_=x8T[c * 128:(c + 1) * 128, :])
                    else:
                        nc.sync.dma_start(out=wqk_sb[:, c, :],
                                          in_=wqk[c * 128:(c + 1) * 128, :])
                    nc.sync.dma_start(out=xt_sb[:, c, :],
                                      in_=xT[c * 128:(c + 1) * 128, :])

                dma_qk_chunk(0)
                nc.sync.dma_start(out=cos_sb, in_=cosT[:, :])
                nc.sync.dma_start(out=sin_sb, in_=sinT[:, :])
                for c in range(1, NE):
                    dma_qk_chunk(c)

                # Q/K projection in transposed [d, s] layout + RoPE.
                # Two j-blocks run jointly c-outer (8 PSUM banks) so the
                # first pair keeps pace with the x DMA stream instead of
                # waiting for all 16 chunks.
                def do_qk_pair(jbs):
                    pts = {}
                    for jb in jbs:
                        for sc in range(SC4):
                            pts[jb, sc] = pp.tile([128, 512], F32, tag="pts",
                                                  name=f"pts_{jb}_{sc}")
                    if use_fp8:
                        for c2 in range(NE2):
                            for jb in jbs:
                                lhsT = wqk_sb[:, c2, :, jb * 128:(jb + 1) * 128]
                                for sc in range(SC4):
                                    nc.tensor.matmul(
                                        pts[jb, sc], lhsT,
                                        x8_sb[:, c2, :, sc * 512:(sc + 1) * 512],
                                        start=(c2 == 0), stop=(c2 == NE2 - 1),
                                        perf_mode=DR,
                                    )
                    else:
                        for c in range(NE):
                            for jb in jbs:
                                lhsT = wqk_sb[:, c, jb * 128:(jb + 1) * 128]
                                for sc in range(SC4):
                                    nc.tensor.matmul(
                                        pts[jb, sc], lhsT,
                                        xt_sb[:, c, sc * 512:(sc + 1) * 512],
                                        start=(c == 0), stop=(c == NE - 1),
                                    )
                    for jb in jbs:
                        for sc in range(SC4):
                            sl = slice(sc * 512, (sc + 1) * 512)
                            xs = rope_t.tile([128, 512], BF, tag="xs")
                            nc.scalar.copy(xs, pts[jb, sc])
                            xw = rope_t.tile([128, 512], BF, tag="xw")
                            nc.sync.dma_start(out=xw[0:64, :], in_=xs[64:128, :])
                            nc.sync.dma_start(out=xw[64:128, :], in_=xs[0:64, :])
                            t1 = rope_t.tile([128, 512], BF, tag="t1")
                            nc.vector.tensor_mul(t1, xs, cos_sb[:, sl])
                            nc.vector.tensor_mul(xw, xw, sin_sb[:, sl])
                            nc.vector.tensor_add(qk_sb[:, jb, sl], t1, xw)

                # V in [t, d] layout (no rope): out[t=128, kvl*128+d]
                def do_v():
                    for st in range(SC16):
                        pv = pp.tile([128, 512], F32, tag="pts", name=f"pv_{st}")
                        for c in range(NE):
                            nc.tensor.matmul(
                                pv[:, 0:HKV * HD],
                                xt_sb[:, c, st * 128:(st + 1) * 128],
                                wv_sb[:, c, :],
                                start=(c == 0), stop=(c == NE - 1),
                            )
                        nc.scalar.copy(v_sb[:, st, :], pv[:, 0:HKV * HD])

                do_qk_pair([HQ, HQ + 1])   # K0, K1
                for c in range(NE):
                    nc.sync.dma_start(out=wv_sb[:, c, :],
                                      in_=wv[c * 128:(c + 1) * 128, :])
                do_qk_pair([0, 1])
                do_qk_pair([2, 3])
                for jb in range(HQ):
                    nc.sync.dma_start(out=wo_sb[:, jb, :],
                                      in_=wo[jb * 128:(jb + 1) * 128, :])
                do_v()

            # ---------------- Phase 2: attention (paired heads) ----------
            with tc.tile_pool(name="pscore", bufs=2, space=bass.MemorySpace.PSUM) as psc, \
                 tc.tile_pool(name="pctx", bufs=2, space=bass.MemorySpace.PSUM) as pcx, \
                 tc.tile_pool(name="pden", bufs=2, space=bass.MemorySpace.PSUM) as pdn, \
                 tc.tile_pool(name="expp", bufs=3) as expp, \
                 tc.tile_pool(name="att", bufs=2) as att:
                for h in range(HQ):
                    kvjb = HQ + h // 2
                    kvl = h // 2
                    for sc in range(SC4):
                        sl = slice(sc * 512, (sc + 1) * 512)
                        cps = pcx.tile([128, 512], F32, tag="cps0", bufs=2, name=f"cps_{h}_{sc}")
                        dps = pdn.tile([1, 512], F32, tag="dps0", bufs=2, name=f"dps_{h}_{sc}")
                        for tcn in range(SC16):
                            sps = psc.tile([128, 512], F32, tag="sps0", bufs=3, name=f"sps_{h}_{sc}_{tcn}")
                            nc.tensor.matmul(
                                sps,
                                qk_sb[:, kvjb, tcn * 128:(tcn + 1) * 128],
                                qk_sb[:, h, sl],
                                start=True, stop=True,
                            )
                            ex = expp.tile([128, 512], BF, tag="ex0")
                            nc.scalar.activation(
                                ex, sps, mybir.ActivationFunctionType.Exp, scale=SCALE
                            )
                            nc.tensor.matmul(
                                cps,
                                v_sb[:, tcn, kvl * 128:(kvl + 1) * 128],
                                ex,
                                start=(tcn == 0), stop=(tcn == SC16 - 1),
                            )
                            nc.tensor.matmul(
                                dps, ones_sb, ex,
                                start=(tcn == 0), stop=(tcn == SC16 - 1),
                            )
                        rc = att.tile([1, 512], F32, tag="rc")
                        nc.vector.reciprocal(rc, dps)
                        rb = att.tile([128, 512], F32, tag="rb")
                        nc.gpsimd.partition_broadcast(rb, rc)
                        nc.vector.tensor_mul(ctx_sb[:, h, sl], cps, rb)

            # ---------------- Phase 3: output projection -----------------
            with tc.tile_pool(name="pout", bufs=2, space=bass.MemorySpace.PSUM) as pou, \
                 tc.tile_pool(name="outs", bufs=2) as outp:
                for so in range(SC16):
                    po = pou.tile([128, 2048], F32, tag="po")
                    for hl in range(HQ):
                        lw = ctx_sb[:, hl, so * 128:(so + 1) * 128]
                        for ec in range(SC4):
                            nc.tensor.matmul(
                                po[:, ec * 512:(ec + 1) * 512],
                                lw,
                                wo_sb[:, hl, ec * 512:(ec + 1) * 512],
                                start=(hl == 0), stop=(hl == HQ - 1),
                            )
                    # drain in 1024-wide halves so the final DMA tail is short
                    ot = outp.tile([128, 2048], BF, tag="ot")
                    for eh in range(2):
                        ehsl = slice(eh * 1024, (eh + 1) * 1024)
                        nc.vector.tensor_copy(ot[:, ehsl], po[:, ehsl])
                        nc.sync.dma_start(
                            out=out[so * 128:(so + 1) * 128, ehsl],
                            in_=ot[:, ehsl])

        if loop_n is not None:
            with tc.For_i(0, loop_n, 1):
                _phases()
        else:
            _phases()

    nc.compile()
    return nc


def _get_nc():
    global _NC
    if _NC is None:
        _NC = _build_program()
    return _NC


def _rope_tables():
    half = HD // 2
    inv_freq = 1.0 / (10000.0 ** (np.arange(half, dtype=np.float64) * 2.0 / HD))
    ang = np.arange(S, dtype=np.float64)[:, None] * inv_freq[None, :]  # (S, 64)
    cos = np.concatenate([np.cos(ang), np.cos(ang)], axis=1).T  # (128, S)
    sin = np.concatenate([-np.sin(ang), np.sin(ang)], axis=1).T  # pre-signed
    return (np.ascontiguousarray(cos).astype(bfnp),
            np.ascontiguousarray(sin).astype(bfnp))


def build_in_maps(x, W_Q, W_K, W_V, W_O, use_fp8=USE_FP8_QK):
    x = np.asarray(x, dtype=np.float32)
    W_Q = np.asarray(W_Q, dtype=np.float32)
    W_K = np.asarray(W_K, dtype=np.float32)
    W_V = np.asarray(W_V, dtype=np.float32)
    W_O = np.asarray(W_O, dtype=np.float32)
    cos, sin = _rope_tables()
    in_maps = []
    xTb = [np.ascontiguousarray(x[b].T).astype(bfnp) for b in range(B)]
    x8Tb = [np.ascontiguousarray(x[b].T).astype(f8np) for b in range(B)]
    for b in range(B):
        for t in range(TP):
            qheads = list(range(HQ * t, HQ * t + HQ))
            kvheads = [HKV * t + i for i in range(HKV)]
            idxq = [d * HEADS + h for h in qheads for d in range(HD)]
            idxkv = [d * KV + kv for kv in kvheads for d in range(HD)]
            rows_o = [h * HD + d for h in qheads for d in range(HD)]
            wqk_np = np.concatenate(
                [W_Q[idxq, :].T, W_K[idxkv, :].T], axis=1)  # (E, 768)
            m = dict(
                xT=xTb[b],
                wv=np.ascontiguousarray(W_V[idxkv, :].T).astype(bfnp),
                wo=np.ascontiguousarray(W_O[:, rows_o].T).astype(bfnp),
                cosT=cos,
                sinT=sin,
            )
            if use_fp8:
                m["x8T"] = x8Tb[b]
                m["wqk"] = np.ascontiguousarray(wqk_np).astype(f8np)
            else:
                m["wqk"] = np.ascontiguousarray(wqk_np).astype(bfnp)
            in_maps.append(m)
    return in_maps


def combine_outs(outs):
    out = np.empty((B, S, EMB), dtype=np.float32)
    for b in range(B):
        acc = outs[TP * b].astype(np.float32).copy()
        for t in range(1, TP):
            acc += outs[TP * b + t]
        out[b] = acc
    return out


LAST_RESULTS = None


def kernel(x, W_Q, W_K, W_V, W_O):
    global LAST_RESULTS
    from concourse.bass_utils import run_bass_kernel_spmd

    nc = _get_nc()
    in_maps = build_in_maps(x, W_Q, W_K, W_V, W_O)
    res = run_bass_kernel_spmd(nc, in_maps, list(range(NCORES)))
    LAST_RESULTS = res
    outs = [r["out"] for r in res.results]
    return combine_outs(outs)


# revision 18
# speedup vs baseline: 1.2746x; 1.2746x over previous
# GQA attention block on 8 Trainium2 NeuronCores.
# Sharding: core = (batch b in {0,1}) x (tensor-parallel t in {0..3}).
# Each core: batch row b, 4 query heads {4t..4t+3}, 2 kv heads {2t, 2t+1}.
# W_Q/W_K/W_V split column-wise (per-head), W_O row-wise; the 4 TP partial
# outputs per batch are summed on the host (the "all-reduce").
#
# vs the original baseline: DMA issue order front-loads the Q/K operands
# (FIFO queue = priority; cos/sin early, wv/wo late, x interleaved with wqk),
# K0/K1 project jointly c-outer so the first pair tracks the x DMA stream,
# rope runs in bf16, out-proj accumulates into 4-bank PSUM tiles drained by
# wide copies, and the output ships as bf16 (summed in fp32 on the host).
# Measured on HW (interleaved A/B, on-device For_i loop-delta): ~454us vs
# ~490us for the baseline in the same session.
# Tried and rejected: fp8 DoubleRow Q/K projections (2.1% rel err alone,
# budget is 2e-2); paired-head attention with lag-1 software pipelining
# (faster in TimelineSim by 28us, ~13us SLOWER on hardware); lag-1 within
# per-head attention (no measurable HW effect).
import math
import sys

sys.path.insert(0, "/opt/trn_rl_repo")

import ml_dtypes
import numpy as np

import concourse.bacc as bacc
import concourse.bass as bass
import concourse.mybir as mybir
import concourse.tile as tile
from contextlib import ExitStack

BF = mybir.dt.bfloat16
F32 = mybir.dt.float32
FP8 = mybir.dt.float8e4
bfnp = ml_dtypes.bfloat16
f8np = mybir.dt.np(FP8)

EMB = 2048
HEADS = 16
G = 2
HD = 128          # head dim
KV = HEADS // G   # 8 kv heads
B = 2
S = 2048
NCORES = 8
TP = 4
HQ = HEADS // TP       # 4 q heads per core
HKV = KV // TP         # 2 kv heads per core
NE = EMB // 128        # 16 contraction chunks
NE2 = EMB // 256       # 8 fp8 DoubleRow chunks
SC4 = S // 512         # 4 s-chunks of 512
SC16 = S // 128        # 16 s-chunks of 128
QKW = (HQ + HKV) * HD  # 768 combined q+k output cols
SCALE = 1.0 / math.sqrt(float(EMB))

USE_FP8_QK = False  # measured: fp8 Q/K alone costs 2.1% rel err (budget 2e-2)

_NC = None


def _build_program(loop_n=None, use_fp8=USE_FP8_QK):
    nc = bacc.Bacc("TRN2", target_bir_lowering=False, debug=False)

    xT = nc.dram_tensor("xT", (EMB, S), BF, kind="ExternalInput")
    if use_fp8:
        x8T = nc.dram_tensor("x8T", (EMB, S), FP8, kind="ExternalInput")
        wqk = nc.dram_tensor("wqk", (EMB, QKW), FP8, kind="ExternalInput")
    else:
        wqk = nc.dram_tensor("wqk", (EMB, QKW), BF, kind="ExternalInput")
    wv = nc.dram_tensor("wv", (EMB, HKV * HD), BF, kind="ExternalInput")
    wo = nc.dram_tensor("wo", (HQ * HD, EMB), BF, kind="ExternalInput")
    cosT = nc.dram_tensor("cosT", (HD, S), BF, kind="ExternalInput")
    sinT = nc.dram_tensor("sinT", (HD, S), BF, kind="ExternalInput")
    out = nc.dram_tensor("out", (S, EMB), BF, kind="ExternalOutput")

    DR = mybir.MatmulPerfMode.DoubleRow

    with tile.TileContext(nc) as tc, ExitStack() as ctx:
        persist = ctx.enter_context(tc.tile_pool(name="persist", bufs=1))
        # qk_sb j-blocks: 0..3 = roped Q heads, 4..5 = roped K kv-heads; [d, s]
        qk_sb = persist.tile([128, HQ + HKV, S], BF)
        # V in [t, d] layout: [t_part, t_chunk, kvl*128+d]
        v_sb = persist.tile([128, SC16, HKV * HD], BF)
        ctx_sb = persist.tile([128, HQ, S], BF)      # [d, head, s]
        wo_sb = persist.tile([128, HQ, EMB], BF)     # [d, head, e_out]
        cos_sb = persist.tile([128, S], BF)
        sin_sb = persist.tile([128, S], BF)
        ones_sb = persist.tile([128, 1], BF)
        warm_a = persist.tile([1, 8], F32)
        warm_b = persist.tile([1, 8], F32)
        nc.vector.memset(ones_sb, 1.0)
        nc.vector.memset(warm_a, 0.0)

        def _phases():
            # preload the exp table set while DMAs stream
            nc.scalar.activation(warm_b, warm_a, mybir.ActivationFunctionType.Exp)

            # ---------------- Phase 1: projections + RoPE ----------------
            with tc.tile_pool(name="xt", bufs=1) as xt_pool, \
                 tc.tile_pool(name="wts", bufs=1) as w_pool, \
                 tc.tile_pool(name="ropet", bufs=3) as rope_t, \
                 tc.tile_pool(name="pproj", bufs=8, space=bass.MemorySpace.PSUM) as pp:
                xt_sb = xt_pool.tile([128, NE, S], BF)
                if use_fp8:
                    x8_sb = xt_pool.tile([128, NE2, 2, S], FP8)
                    wqk_sb = w_pool.tile([128, NE2, 2, QKW], FP8)
                else:
                    wqk_sb = w_pool.tile([128, NE, QKW], BF)
                wv_sb = w_pool.tile([128, NE, HKV * HD], BF)

                # DMA order (the queue is FIFO, so order = priority):
                # chunk-0 q/k operands, cos/sin, remaining q/k chunks.
                # wv is emitted after the K pair, wo after all projections,
                # and each rope's swap DMAs interleave naturally.
                def dma_qk_chunk(c):
                    if use_fp8:
                        c2, ko = divmod(c, 2)
                        nc.sync.dma_start(out=wqk_sb[:, c2, ko, :],
                                          in_=wqk[c * 128:(c + 1) * 128, :])
                        nc.sync.dma_start(out=x8_sb[:, c2, ko, :],
                                          in_=x8T[c * 128:(c + 1) * 128, :])
                    else:
                        nc.sync.dma_start(out=wqk_sb[:, c, :],
                                          in_=wqk[c * 128:(c + 1) * 128, :])
                    nc.sync.dma_start(out=xt_sb[:, c, :],
                                      in_=xT[c * 128:(c + 1) * 128, :])

                dma_qk_chunk(0)
                nc.sync.dma_start(out=cos_sb, in_=cosT[:, :])
                nc.sync.dma_start(out=sin_sb, in_=sinT[:, :])
                for c in range(1, NE):
                    dma_qk_chunk(c)

                # Q/K projection in transposed [d, s] layout + RoPE.
                # Two j-blocks run jointly c-outer (8 PSUM banks) so the
                # first pair keeps pace with the x DMA stream instead of
                # waiting for all 16 chunks.
                def do_qk_pair(jbs):
                    pts = {}
                    for jb in jbs:
                        for sc in range(SC4):
                            pts[jb, sc] = pp.tile([128, 512], F32, tag="pts",
                                                  name=f"pts_{jb}_{sc}")
                    if use_fp8:
                        for c2 in range(NE2):
                            for jb in jbs:
                                lhsT = wqk_sb[:, c2, :, jb * 128:(jb + 1) * 128]
                                for sc in range(SC4):
                                    nc.tensor.matmul(
                                        pts[jb, sc], lhsT,
                                        x8_sb[:, c2, :, sc * 512:(sc + 1) * 512],
                                        start=(c2 == 0), stop=(c2 == NE2 - 1),
                                        perf_mode=DR,
                                    )
                    else:
                        for c in range(NE):
                            for jb in jbs:
                                lhsT = wqk_sb[:, c, jb * 128:(jb + 1) * 128]
                                for sc in range(SC4):
                                    nc.tensor.matmul(
                                        pts[jb, sc], lhsT,
                                        xt_sb[:, c, sc * 512:(sc + 1) * 512],
                                        start=(c == 0), stop=(c == NE - 1),
                                    )
                    for jb in jbs:
                        for sc in range(SC4):
                            sl = slice(sc * 512, (sc + 1) * 512)
                            xs = rope_t.tile([128, 512], BF, tag="xs")
                            nc.scalar.copy(xs, pts[jb, sc])
                            xw = rope_t.tile([128, 512], BF, tag="xw")
                            nc.sync.dma_start(out=xw[0:64, :], in_=xs[64:128, :])
                            nc.sync.dma_start(out=xw[64:128, :], in_=xs[0:64, :])
                            t1 = rope_t.tile([128, 512], BF, tag="t1")
                            nc.vector.tensor_mul(t1, xs, cos_sb[:, sl])
                            nc.vector.tensor_mul(xw, xw, sin_sb[:, sl])
                            nc.vector.tensor_add(qk_sb[:, jb, sl], t1, xw)

                # V in [t, d] layout (no rope): out[t=128, kvl*128+d]
                def do_v():
                    for st in range(SC16):
                        pv = pp.tile([128, 512], F32, tag="pts", name=f"pv_{st}")
                        for c in range(NE):
                            nc.tensor.matmul(
                                pv[:, 0:HKV * HD],
                                xt_sb[:, c, st * 128:(st + 1) * 128],
                                wv_sb[:, c, :],
                                start=(c == 0), stop=(c == NE - 1),
                            )
                        nc.scalar.copy(v_sb[:, st, :], pv[:, 0:HKV * HD])

                do_qk_pair([HQ, HQ + 1])   # K0, K1
                for c in range(NE):
                    nc.sync.dma_start(out=wv_sb[:, c, :],
                                      in_=wv[c * 128:(c + 1) * 128, :])
                do_qk_pair([0, 1])
                do_qk_pair([2, 3])
                for jb in range(HQ):
                    nc.sync.dma_start(out=wo_sb[:, jb, :],
                                      in_=wo[jb * 128:(jb + 1) * 128, :])
                do_v()

            # ---------------- Phase 2: attention (paired heads) ----------
            with tc.tile_pool(name="pscore", bufs=2, space=bass.MemorySpace.PSUM) as psc, \
                 tc.tile_pool(name="pctx", bufs=2, space=bass.MemorySpace.PSUM) as pcx, \
                 tc.tile_pool(name="pden", bufs=2, space=bass.MemorySpace.PSUM) as pdn, \
                 tc.tile_pool(name="expp", bufs=3) as expp, \
                 tc.tile_pool(name="att", bufs=2) as att:
                for h in range(HQ):
                    kvjb = HQ + h // 2
                    kvl = h // 2
                    for sc in range(SC4):
                        sl = slice(sc * 512, (sc + 1) * 512)
                        cps = pcx.tile([128, 512], F32, tag="cps0", bufs=2, name=f"cps_{h}_{sc}")
                        dps = pdn.tile([1, 512], F32, tag="dps0", bufs=2, name=f"dps_{h}_{sc}")
                        for tcn in range(SC16):
                            sps = psc.tile([128, 512], F32, tag="sps0", bufs=3, name=f"sps_{h}_{sc}_{tcn}")
                            nc.tensor.matmul(
                                sps,
                                qk_sb[:, kvjb, tcn * 128:(tcn + 1) * 128],
                                qk_sb[:, h, sl],
                                start=True, stop=True,
                            )
                            ex = expp.tile([128, 512], BF, tag="ex0")
                            nc.scalar.activation(
                                ex, sps, mybir.ActivationFunctionType.Exp, scale=SCALE
                            )
                            nc.tensor.matmul(
                                cps,
                                v_sb[:, tcn, kvl * 128:(kvl + 1) * 128],
                                ex,
                                start=(tcn == 0), stop=(tcn == SC16 - 1),
                            )
                            nc.tensor.matmul(
                                dps, ones_sb, ex,
                                start=(tcn == 0), stop=(tcn == SC16 - 1),
                            )
                        rc = att.tile([1, 512], F32, tag="rc")
                        nc.vector.reciprocal(rc, dps)
                        rb = att.tile([128, 512], F32, tag="rb")
                        nc.gpsimd.partition_broadcast(rb, rc)
                        nc.vector.tensor_mul(ctx_sb[:, h, sl], cps, rb)

            # ---------------- Phase 3: output projection -----------------
            with tc.tile_pool(name="pout", bufs=2, space=bass.MemorySpace.PSUM) as pou, \
                 tc.tile_pool(name="outs", bufs=2) as outp:
                for so in range(SC16):
                    po = pou.tile([128, 2048], F32, tag="po")
                    for hl in range(HQ):
                        lw = ctx_sb[:, hl, so * 128:(so + 1) * 128]
                        for ec in range(SC4):
                            nc.tensor.matmul(
                                po[:, ec * 512:(ec + 1) * 512],
                                lw,
                                wo_sb[:, hl, ec * 512:(ec + 1) * 512],
                                start=(hl == 0), stop=(hl == HQ - 1),
                            )
                    # drain in 1024-wide halves so the final DMA tail is short
                    ot = outp.tile([128, 2048], BF, tag="ot")
                    for eh in range(2):
                        ehsl = slice(eh * 1024, (eh + 1) * 1024)
                        nc.vector.tensor_copy(ot[:, ehsl], po[:, ehsl])
                        nc.sync.dma_start(
                            out=out[so * 128:(so + 1) * 128, ehsl],
                            in_=ot[:, ehsl])

        if loop_n is not None:
            with tc.For_i(0, loop_n, 1):
                _phases()
        else:
            _phases()

    nc.compile()
    return nc


def _get_nc():
    global _NC
    if _NC is None:
        _NC = _build_program()
    return _NC


def _rope_tables():
    half = HD // 2
    inv_freq = 1.0 / (10000.0 ** (np.arange(half, dtype=np.float64) * 2.0 / HD))
    ang = np.arange(S, dtype=np.float64)[:, None] * inv_freq[None, :]  # (S, 64)
    cos = np.concatenate([np.cos(ang), np.cos(ang)], axis=1).T  # (128, S)
    sin = np.concatenate([-np.sin(ang), np.sin(ang)], axis=1).T  # pre-signed
    return (np.ascontiguousarray(cos).astype(bfnp),
            np.ascontiguousarray(sin).astype(bfnp))


def build_in_maps(x, W_Q, W_K, W_V, W_O, use_fp8=USE_FP8_QK):
    x = np.asarray(x, dtype=np.float32)
    W_Q = np.asarray(W_Q, dtype=np.float32)
    W_K = np.asarray(W_K, dtype=np.float32)
    W_V = np.asarray(W_V, dtype=np.float32)
    W_O = np.asarray(W_O, dtype=np.float32)
    cos, sin = _rope_tables()
    in_maps = []
    xTb = [np.ascontiguousarray(x[b].T).astype(bfnp) for b in range(B)]
    x8Tb = [np.ascontiguousarray(x[b].T).astype(f8np) for b in range(B)]
    for b in range(B):
        for t in range(TP):
            qheads = list(range(HQ * t, HQ * t + HQ))
            kvheads = [HKV * t + i for i in range(HKV)]
            idxq = [d * HEADS + h for h in qheads for d in range(HD)]
            idxkv = [d * KV + kv for kv in kvheads for d in range(HD)]
            rows_o = [h * HD + d for h in qheads for d in range(HD)]
            wqk_np = np.concatenate(
                [W_Q[idxq, :].T, W_K[idxkv, :].T], axis=1)  # (E, 768)
            m = dict(
                xT=xTb[b],
                wv=np.ascontiguousarray(W_V[idxkv, :].T).astype(bfnp),
                wo=np.ascontiguousarray(W_O[:, rows_o].T).astype(bfnp),
                cosT=cos,
                sinT=sin,
            )
            if use_fp8:
                m["x8T"] = x8Tb[b]
                m["wqk"] = np.ascontiguousarray(wqk_np).astype(f8np)
            else:
                m["wqk"] = np.ascontiguousarray(wqk_np).astype(bfnp)
            in_maps.append(m)
    return in_maps


def combine_outs(outs):
    out = np.empty((B, S, EMB), dtype=np.float32)
    for b in range(B):
        acc = outs[TP * b].astype(np.float32).copy()
        for t in range(1, TP):
            acc += outs[TP * b + t]
        out[b] = acc
    return out


LAST_RESULTS = None


def kernel(x, W_Q, W_K, W_V, W_O):
    global LAST_RESULTS
    from concourse.bass_utils import run_bass_kernel_spmd

    nc = _get_nc()
    in_maps = build_in_maps(x, W_Q, W_K, W_V, W_O)
    res = run_bass_kernel_spmd(nc, in_maps, list(range(NCORES)))
    LAST_RESULTS = res
    outs = [r["out"] for r in res.results]
    return combine_outs(outs)


# revision 19
# speedup vs baseline: 1.2983x; 1.0186x over previous
# GQA attention block on 8 Trainium2 NeuronCores.
# Sharding: core = (batch b in {0,1}) x (tensor-parallel t in {0..3}).
# Each core: batch row b, 4 query heads {4t..4t+3}, 2 kv heads {2t, 2t+1}.
# W_Q/W_K/W_V split column-wise (per-head), W_O row-wise; the 4 TP partial
# outputs per batch are summed on the host (the "all-reduce").
#
# vs the original baseline: DMA issue order front-loads the Q/K operands
# (FIFO queue = priority; cos/sin early, wv/wo late, x interleaved with wqk),
# K0/K1 project jointly c-outer so the first pair tracks the x DMA stream,
# rope runs in bf16, out-proj accumulates into 4-bank PSUM tiles drained by
# wide copies, and the output ships as bf16 (summed in fp32 on the host).
# Measured on HW (interleaved A/B, on-device For_i loop-delta): ~454us vs
# ~490us for the baseline in the same session.
# Tried and rejected: fp8 DoubleRow Q/K projections (2.1% rel err alone,
# budget is 2e-2); paired-head attention with lag-1 software pipelining
# (faster in TimelineSim by 28us, ~13us SLOWER on hardware); lag-1 within
# per-head attention (no measurable HW effect).
import math
import sys

sys.path.insert(0, "/opt/trn_rl_repo")

import ml_dtypes
import numpy as np

import concourse.bacc as bacc
import concourse.bass as bass
import concourse.mybir as mybir
import concourse.tile as tile
from contextlib import ExitStack

BF = mybir.dt.bfloat16
F32 = mybir.dt.float32
FP8 = mybir.dt.float8e4
bfnp = ml_dtypes.bfloat16
f8np = mybir.dt.np(FP8)

EMB = 2048
HEADS = 16
G = 2
HD = 128          # head dim
KV = HEADS // G   # 8 kv heads
B = 2
S = 2048
NCORES = 8
TP = 4
HQ = HEADS // TP       # 4 q heads per core
HKV = KV // TP         # 2 kv heads per core
NE = EMB // 128        # 16 contraction chunks
NE2 = EMB // 256       # 8 fp8 DoubleRow chunks
SC4 = S // 512         # 4 s-chunks of 512
SC16 = S // 128        # 16 s-chunks of 128
QKW = (HQ + HKV) * HD  # 768 combined q+k output cols
SCALE = 1.0 / math.sqrt(float(EMB))

USE_FP8_QK = False  # measured: fp8 Q/K alone costs 2.1% rel err (budget 2e-2)

_NC = None


def _build_program(loop_n=None, use_fp8=USE_FP8_QK):
    nc = bacc.Bacc("TRN2", target_bir_lowering=False, debug=False)

    xT = nc.dram_tensor("xT", (EMB, S), BF, kind="ExternalInput")
    if use_fp8:
        x8T = nc.dram_tensor("x8T", (EMB, S), FP8, kind="ExternalInput")
        wqk = nc.dram_tensor("wqk", (EMB, QKW), FP8, kind="ExternalInput")
    else:
        wqk = nc.dram_tensor("wqk", (EMB, QKW), BF, kind="ExternalInput")
    wv = nc.dram_tensor("wv", (EMB, HKV * HD), BF, kind="ExternalInput")
    wo = nc.dram_tensor("wo", (HQ * HD, EMB), BF, kind="ExternalInput")
    cosT = nc.dram_tensor("cosT", (HD, S), BF, kind="ExternalInput")
    sinT = nc.dram_tensor("sinT", (HD, S), BF, kind="ExternalInput")
    out = nc.dram_tensor("out", (S, EMB), BF, kind="ExternalOutput")

    DR = mybir.MatmulPerfMode.DoubleRow

    with tile.TileContext(nc) as tc, ExitStack() as ctx:
        persist = ctx.enter_context(tc.tile_pool(name="persist", bufs=1))
        # qk_sb j-blocks: 0..3 = roped Q heads, 4..5 = roped K kv-heads; [d, s]
        qk_sb = persist.tile([128, HQ + HKV, S], BF)
        # V in [t, d] layout: [t_part, t_chunk, kvl*128+d]
        v_sb = persist.tile([128, SC16, HKV * HD], BF)
        ctx_sb = persist.tile([128, HQ, S], BF)      # [d, head, s]
        wo_sb = persist.tile([128, HQ, EMB], BF)     # [d, head, e_out]
        cos_sb = persist.tile([128, S], BF)
        sin_sb = persist.tile([128, S], BF)
        ones_sb = persist.tile([128, 1], BF)
        warm_a = persist.tile([1, 8], F32)
        warm_b = persist.tile([1, 8], F32)
        nc.vector.memset(ones_sb, 1.0)
        nc.vector.memset(warm_a, 0.0)

        def _phases():
            # preload the exp table set while DMAs stream
            nc.scalar.activation(warm_b, warm_a, mybir.ActivationFunctionType.Exp)

            # ---------------- Phase 1: projections + RoPE ----------------
            with tc.tile_pool(name="xt", bufs=1) as xt_pool, \
                 tc.tile_pool(name="wts", bufs=1) as w_pool, \
                 tc.tile_pool(name="ropet", bufs=3) as rope_t, \
                 tc.tile_pool(name="pproj", bufs=8, space=bass.MemorySpace.PSUM) as pp:
                xt_sb = xt_pool.tile([128, NE, S], BF)
                if use_fp8:
                    x8_sb = xt_pool.tile([128, NE2, 2, S], FP8)
                    wqk_sb = w_pool.tile([128, NE2, 2, QKW], FP8)
                else:
                    wqk_sb = w_pool.tile([128, NE, QKW], BF)
                wv_sb = w_pool.tile([128, NE, HKV * HD], BF)

                # DMA order (the queue is FIFO, so order = priority):
                # chunk-0 q/k operands, cos/sin, remaining q/k chunks.
                # wv is emitted after the K pair, wo after all projections,
                # and each rope's swap DMAs interleave naturally.
                def dma_qk_chunk(c):
                    if use_fp8:
                        c2, ko = divmod(c, 2)
                        nc.sync.dma_start(out=wqk_sb[:, c2, ko, :],
                                          in_=wqk[c * 128:(c + 1) * 128, :])
                        nc.sync.dma_start(out=x8_sb[:, c2, ko, :],
                                          in_=x8T[c * 128:(c + 1) * 128, :])
                    else:
                        nc.sync.dma_start(out=wqk_sb[:, c, :],
                                          in_=wqk[c * 128:(c + 1) * 128, :])
                    nc.sync.dma_start(out=xt_sb[:, c, :],
                                      in_=xT[c * 128:(c + 1) * 128, :])
                    nc.sync.dma_start(out=wv_sb[:, c, :],
                                      in_=wv[c * 128:(c + 1) * 128, :])

                dma_qk_chunk(0)
                nc.sync.dma_start(out=cos_sb, in_=cosT[:, :])
                nc.sync.dma_start(out=sin_sb, in_=sinT[:, :])
                for c in range(1, NE):
                    dma_qk_chunk(c)

                # Q/K projection in transposed [d, s] layout + RoPE.
                # Two j-blocks run jointly c-outer (8 PSUM banks) so the
                # first pair keeps pace with the x DMA stream instead of
                # waiting for all 16 chunks.
                def do_qk_pair(jbs):
                    pts = {}
                    for jb in jbs:
                        for sc in range(SC4):
                            pts[jb, sc] = pp.tile([128, 512], F32, tag="pts",
                                                  name=f"pts_{jb}_{sc}")
                    if use_fp8:
                        for c2 in range(NE2):
                            for jb in jbs:
                                lhsT = wqk_sb[:, c2, :, jb * 128:(jb + 1) * 128]
                                for sc in range(SC4):
                                    nc.tensor.matmul(
                                        pts[jb, sc], lhsT,
                                        x8_sb[:, c2, :, sc * 512:(sc + 1) * 512],
                                        start=(c2 == 0), stop=(c2 == NE2 - 1),
                                        perf_mode=DR,
                                    )
                    else:
                        for c in range(NE):
                            for jb in jbs:
                                lhsT = wqk_sb[:, c, jb * 128:(jb + 1) * 128]
                                for sc in range(SC4):
                                    nc.tensor.matmul(
                                        pts[jb, sc], lhsT,
                                        xt_sb[:, c, sc * 512:(sc + 1) * 512],
                                        start=(c == 0), stop=(c == NE - 1),
                                    )
                    for jb in jbs:
                        for sc in range(SC4):
                            sl = slice(sc * 512, (sc + 1) * 512)
                            xs = rope_t.tile([128, 512], BF, tag="xs")
                            nc.scalar.copy(xs, pts[jb, sc])
                            xw = rope_t.tile([128, 512], BF, tag="xw")
                            nc.sync.dma_start(out=xw[0:64, :], in_=xs[64:128, :])
                            nc.sync.dma_start(out=xw[64:128, :], in_=xs[0:64, :])
                            t1 = rope_t.tile([128, 512], BF, tag="t1")
                            nc.vector.tensor_mul(t1, xs, cos_sb[:, sl])
                            nc.vector.tensor_mul(xw, xw, sin_sb[:, sl])
                            nc.vector.tensor_add(qk_sb[:, jb, sl], t1, xw)

                # One Q head block + 4 V st-chunks (V in [t, d] layout,
                # no rope), Q and V matmuls alternating per c-chunk.
                def do_q_with_v(jb, vsts):
                    pts = {}
                    for sc in range(SC4):
                        pts[sc] = pp.tile([128, 512], F32, tag="pts",
                                          name=f"pts_{jb}_{sc}")
                    pvs = {}
                    for st in vsts:
                        pvs[st] = pp.tile([128, 512], F32, tag="pts",
                                          name=f"pv_{st}")
                    for c in range(NE):
                        lhsT = wqk_sb[:, c, jb * 128:(jb + 1) * 128]
                        for k in range(SC4):
                            nc.tensor.matmul(
                                pts[k], lhsT,
                                xt_sb[:, c, k * 512:(k + 1) * 512],
                                start=(c == 0), stop=(c == NE - 1),
                            )
                            st = vsts[k]
                            nc.tensor.matmul(
                                pvs[st][:, 0:HKV * HD],
                                xt_sb[:, c, st * 128:(st + 1) * 128],
                                wv_sb[:, c, :],
                                start=(c == 0), stop=(c == NE - 1),
                            )
                    for sc in range(SC4):
                        sl = slice(sc * 512, (sc + 1) * 512)
                        xs = rope_t.tile([128, 512], BF, tag="xs")
                        nc.scalar.copy(xs, pts[sc])
                        xw = rope_t.tile([128, 512], BF, tag="xw")
                        nc.sync.dma_start(out=xw[0:64, :], in_=xs[64:128, :])
                        nc.sync.dma_start(out=xw[64:128, :], in_=xs[0:64, :])
                        t1 = rope_t.tile([128, 512], BF, tag="t1")
                        nc.vector.tensor_mul(t1, xs, cos_sb[:, sl])
                        nc.vector.tensor_mul(xw, xw, sin_sb[:, sl])
                        nc.vector.tensor_add(qk_sb[:, jb, sl], t1, xw)
                    for st in vsts:
                        nc.scalar.copy(v_sb[:, st, :], pvs[st][:, 0:HKV * HD])

                do_qk_pair([HQ, HQ + 1])   # K0, K1
                # Each Q block carries 4 V-proj st-chunks, alternating Q and
                # V matmuls: every V ldweights (107ns, one per matmul) hides
                # under a 213ns Q stream instead of under another 107ns V
                # matmul with zero slack.
                do_q_with_v(0, [0, 1, 2, 3])
                do_q_with_v(1, [4, 5, 6, 7])
                do_q_with_v(2, [8, 9, 10, 11])
                for jb in range(HQ):
                    nc.sync.dma_start(out=wo_sb[:, jb, :],
                                      in_=wo[jb * 128:(jb + 1) * 128, :])
                do_q_with_v(3, [12, 13, 14, 15])

            # ---------------- Phase 2: attention (paired heads) ----------
            with tc.tile_pool(name="pscore", bufs=2, space=bass.MemorySpace.PSUM) as psc, \
                 tc.tile_pool(name="pctx", bufs=2, space=bass.MemorySpace.PSUM) as pcx, \
                 tc.tile_pool(name="pden", bufs=2, space=bass.MemorySpace.PSUM) as pdn, \
                 tc.tile_pool(name="expp", bufs=3) as expp, \
                 tc.tile_pool(name="att", bufs=2) as att:
                for h in range(HQ):
                    kvjb = HQ + h // 2
                    kvl = h // 2
                    for sc in range(SC4):
                        sl = slice(sc * 512, (sc + 1) * 512)
                        cps = pcx.tile([128, 512], F32, tag="cps0", bufs=2, name=f"cps_{h}_{sc}")
                        dps = pdn.tile([1, 512], F32, tag="dps0", bufs=2, name=f"dps_{h}_{sc}")
                        for tcn in range(SC16):
                            sps = psc.tile([128, 512], F32, tag="sps0", bufs=3, name=f"sps_{h}_{sc}_{tcn}")
                            nc.tensor.matmul(
                                sps,
                                qk_sb[:, kvjb, tcn * 128:(tcn + 1) * 128],
                                qk_sb[:, h, sl],
                                start=True, stop=True,
                            )
                            ex = expp.tile([128, 512], BF, tag="ex0")
                            nc.scalar.activation(
                                ex, sps, mybir.ActivationFunctionType.Exp, scale=SCALE
                            )
                            nc.tensor.matmul(
                                cps,
                                v_sb[:, tcn, kvl * 128:(kvl + 1) * 128],
                                ex,
                                start=(tcn == 0), stop=(tcn == SC16 - 1),
                            )
                            nc.tensor.matmul(
                                dps, ones_sb, ex,
                                start=(tcn == 0), stop=(tcn == SC16 - 1),
                            )
                        rc = att.tile([1, 512], F32, tag="rc")
                        nc.vector.reciprocal(rc, dps)
                        rb = att.tile([128, 512], F32, tag="rb")
                        nc.gpsimd.partition_broadcast(rb, rc)
                        nc.vector.tensor_mul(ctx_sb[:, h, sl], cps, rb)

            # ---------------- Phase 3: output projection -----------------
            with tc.tile_pool(name="pout", bufs=2, space=bass.MemorySpace.PSUM) as pou, \
                 tc.tile_pool(name="outs", bufs=2) as outp:
                for so in range(SC16):
                    po = pou.tile([128, 2048], F32, tag="po")
                    for hl in range(HQ):
                        lw = ctx_sb[:, hl, so * 128:(so + 1) * 128]
                        for ec in range(SC4):
                            nc.tensor.matmul(
                                po[:, ec * 512:(ec + 1) * 512],
                                lw,
                                wo_sb[:, hl, ec * 512:(ec + 1) * 512],
                                start=(hl == 0), stop=(hl == HQ - 1),
                            )
                    # drain in 1024-wide halves so the final DMA tail is short
                    ot = outp.tile([128, 2048], BF, tag="ot")
                    for eh in range(2):
                        ehsl = slice(eh * 1024, (eh + 1) * 1024)
                        nc.vector.tensor_copy(ot[:, ehsl], po[:, ehsl])
                        nc.sync.dma_start(
                            out=out[so * 128:(so + 1) * 128, ehsl],
                            in_=ot[:, ehsl])

        if loop_n is not None:
            with tc.For_i(0, loop_n, 1):
                _phases()
        else:
            _phases()

    nc.compile()
    return nc


def _get_nc():
    global _NC
    if _NC is None:
        _NC = _build_program()
    return _NC


def _rope_tables():
    half = HD // 2
    inv_freq = 1.0 / (10000.0 ** (np.arange(half, dtype=np.float64) * 2.0 / HD))
    ang = np.arange(S, dtype=np.float64)[:, None] * inv_freq[None, :]  # (S, 64)
    cos = np.concatenate([np.cos(ang), np.cos(ang)], axis=1).T  # (128, S)
    sin = np.concatenate([-np.sin(ang), np.sin(ang)], axis=1).T  # pre-signed
    return (np.ascontiguousarray(cos).astype(bfnp),
            np.ascontiguousarray(sin).astype(bfnp))


def build_in_maps(x, W_Q, W_K, W_V, W_O, use_fp8=USE_FP8_QK):
    x = np.asarray(x, dtype=np.float32)
    W_Q = np.asarray(W_Q, dtype=np.float32)
    W_K = np.asarray(W_K, dtype=np.float32)
    W_V = np.asarray(W_V, dtype=np.float32)
    W_O = np.asarray(W_O, dtype=np.float32)
    cos, sin = _rope_tables()
    in_maps = []
    xTb = [np.ascontiguousarray(x[b].T).astype(bfnp) for b in range(B)]
    x8Tb = [np.ascontiguousarray(x[b].T).astype(f8np) for b in range(B)]
    for b in range(B):
        for t in range(TP):
            qheads = list(range(HQ * t, HQ * t + HQ))
            kvheads = [HKV * t + i for i in range(HKV)]
            idxq = [d * HEADS + h for h in qheads for d in range(HD)]
            idxkv = [d * KV + kv for kv in kvheads for d in range(HD)]
            rows_o = [h * HD + d for h in qheads for d in range(HD)]
            wqk_np = np.concatenate(
                [W_Q[idxq, :].T, W_K[idxkv, :].T], axis=1)  # (E, 768)
            m = dict(
                xT=xTb[b],
                wv=np.ascontiguousarray(W_V[idxkv, :].T).astype(bfnp),
                wo=np.ascontiguousarray(W_O[:, rows_o].T).astype(bfnp),
                cosT=cos,
                sinT=sin,
            )
            if use_fp8:
                m["x8T"] = x8Tb[b]
                m["wqk"] = np.ascontiguousarray(wqk_np).astype(f8np)
            else:
                m["wqk"] = np.ascontiguousarray(wqk_np).astype(bfnp)
            in_maps.append(m)
    return in_maps


def combine_outs(outs):
    out = np.empty((B, S, EMB), dtype=np.float32)
    for b in range(B):
        acc = outs[TP * b].astype(np.float32).copy()
        for t in range(1, TP):
            acc += outs[TP * b + t]
        out[b] = acc
    return out


LAST_RESULTS = None


def kernel(x, W_Q, W_K, W_V, W_O):
    global LAST_RESULTS
    from concourse.bass_utils import run_bass_kernel_spmd

    nc = _get_nc()
    in_maps = build_in_maps(x, W_Q, W_K, W_V, W_O)
    res = run_bass_kernel_spmd(nc, in_maps, list(range(NCORES)))
    LAST_RESULTS = res
    outs = [r["out"] for r in res.results]
    return combine_outs(outs)
